# revision 1
# baseline (speedup 1.0000x reference)
"""TextCNN-style conv layer (kernel sizes 3/4/5, EMB=300 -> DEPTH=256, bias,
ReLU, max-pool over time) as a Bass/Tile kernel for 8 Trainium2 NeuronCores.

Strategy: data-parallel over batch (8 samples per core), weights replicated.

Conv as dense-K matmuls: for branch n, window output
y[d, i] = sum_{k < n*300} Xrep[k, i] * Wn[d, k]  with  Xrep[k, i] =
x[i + k//300, k%300] -- the im2col matrix.  Xrep rows are materialized in
SBUF as 12 K-tiles of 128 rows per sample, built by <=2 shifted DMA segments
per tile straight from the transposed input in DRAM (a row (j, e) is just
x_t[e, j:] -- a free-dim offset), so no host-side replication and each
branch contracts over ceil(n*300/128) dense K=128 tiles (8/10/12 -> 30
matmuls per sample per depth-half vs 36 for the per-(j,chunk) split).
Branch boundaries that fall inside a tile are handled by zero-padding the
*weights* (the x rows there hold valid shifted data).  The final K-tile's
rows past 1500 are never written, so its matmuls contract only K=92.

dtype float32r: FP22 multiplies at full PE rate, fp32 PSUM accumulate; the
moving free-dim count must be even, so branches with odd SEQ-n compute one
extra (still valid) window that the max-reduce then ignores.

Epilogue: relu(max_i(y + b)) == max(0, max_i y + b): DVE reduce_max over the
window axis straight out of PSUM, broadcast bias add + clamp at 0, output
staged [d, branch, half, sample] per core and de-transposed on host.
"""

import numpy as np

B, SEQ, EMB = 64, 394, 300
DEPTH = 256
NCORES = 8
BPC = B // NCORES  # samples per core
SEQP = 400  # x_t free-dim padded (zeros) so shifted loads stay in bounds
NS = (3, 4, 5)
NTILES = (8, 10, 12)  # ceil(n*300/128) K-tiles per branch
COLB = (0, 8, 18)  # weight column base per branch
NCOL = 30
KTOT = 12  # distinct Xrep K-tiles per sample

# DMA segments building the 12 Xrep K-tiles: (tile r, p0, plen, j, e0)
_SEGS = []
for _r in range(KTOT):
    _k, _k1 = 128 * _r, min(128 * (_r + 1), 5 * EMB)
    while _k < _k1:
        _j, _e = divmod(_k, EMB)
        _plen = min(_k1 - _k, EMB - _e)
        _SEGS.append((_r, _k - 128 * _r, _plen, _j, _e))
        _k += _plen

TRACE = False
LAST_RESULT = None

_built = None


def _build_bass():
    import concourse.mybir as mybir
    import concourse.tile as tile
    from concourse import bacc
    from contextlib import ExitStack

    f32 = mybir.dt.float32
    f32r = mybir.dt.float32r
    f16 = mybir.dt.float16

    nc = bacc.Bacc("TRN2", target_bir_lowering=False)
    xt_d = nc.dram_tensor("xt", (BPC, KTOT, 128, SEQP), f16, kind="ExternalInput")
    wq_d = nc.dram_tensor("wq", (128, 2, NCOL, 128), f16, kind="ExternalInput")
    bp_d = nc.dram_tensor("bp", (128, 3, 2), f32, kind="ExternalInput")
    out_d = nc.dram_tensor("out_t", (128, 3, 2, BPC), f32, kind="ExternalOutput")

    with tile.TileContext(nc) as tc, ExitStack() as ctx:
        xpool = ctx.enter_context(tc.tile_pool(name="x", bufs=5))
        wpool = ctx.enter_context(tc.tile_pool(name="w", bufs=1))
        cpool = ctx.enter_context(tc.tile_pool(name="consts", bufs=1))
        spool = ctx.enter_context(tc.tile_pool(name="stage", bufs=1))
        pspool = ctx.enter_context(tc.tile_pool(name="ps", bufs=8, space="PSUM"))

        # x segments alternate over the two fast HWDGE rings (SP, ACT);
        # weights + bias go on the gpsimd SWDGE ring in parallel so they
        # never block the x pipeline.
        hw_engines = (nc.sync, nc.scalar)
        rr = [0]

        wts = {}

        def load_w(dh, br, eng):
            nt = NTILES[br]
            wt = wpool.tile([128, nt, 128], f16, tag=f"w{dh}{br}")
            eng.dma_start(wt[:], wq_d[:, dh, COLB[br] : COLB[br] + nt, :])
            wts[dh, br] = wt

        def load_x(s):
            # One pool tile + one contiguous DMA per K-tile: a matmul waits
            # only on the single DMA that wrote its contraction rows.
            xr = [
                xpool.tile([128, SEQP], f16, tag=f"x{r}", name=f"x{r}_{s}")
                for r in range(KTOT)
            ]
            for r in range(KTOT):
                eng = hw_engines[rr[0] % 2]
                rr[0] += 1
                eng.dma_start(xr[r][:], xt_d[s, r])
            return xr

        # The whole working set (3.9MB weights + 2.4MB im2col per sample)
        # drains from HBM at ~350GB/s, so the first ~25us are DMA-paced.
        # Interleave the loads in need-order and run samples 0-1 group-major
        # (stretching each weight tile's deadline) before switching to
        # sample-major for the pipelined steady state.
        load_w(0, 0, nc.sync)
        xrs = [load_x(0)]
        load_w(0, 1, nc.scalar)
        load_w(0, 2, nc.sync)
        load_w(1, 0, nc.scalar)
        load_w(1, 1, nc.sync)
        load_w(1, 2, nc.scalar)
        xrs.append(load_x(1))
        bt = cpool.tile([128, 3, 2], f32)
        nc.gpsimd.dma_start(bt[:], bp_d[:])
        xrs.append(load_x(2))

        stage = spool.tile([128, 3, 2, BPC], f32)

        def do_group(s, dh, br):
            n = NS[br]
            nw = SEQ - n  # windows the reference maxes over
            nmm = nw + (nw & 1)  # keep the moving count even
            nt = NTILES[br]
            xr = xrs[s]
            ps = pspool.tile([128, 512], f32, tag="ps", name=f"ps_{s}_{dh}_{br}")
            for r in range(nt):
                kk = min(128, 5 * EMB - 128 * r)  # 92 on the last tile
                nc.tensor.matmul(
                    ps[:, :nmm],
                    lhsT=wts[dh, br][:kk, r, :],
                    rhs=xr[r][:kk, :nmm],
                    start=(r == 0),
                    stop=(r == nt - 1),
                )
            nc.vector.reduce_max(
                stage[:, br, dh, s : s + 1],
                ps[:, :nw],
                axis=mybir.AxisListType.X,
            )

        for s in range(BPC):
            if 3 <= s + 3 < BPC + 3 and s + 3 < BPC:
                xrs.append(load_x(s + 3))
            for dh in range(2):
                for br in range(3):
                    do_group(s, dh, br)

        stage2 = spool.tile([128, 3, 2, BPC], f32)
        nc.vector.tensor_tensor(
            stage2[:],
            stage[:],
            bt[:, :, :, None].to_broadcast((128, 3, 2, BPC)),
            mybir.AluOpType.add,
        )
        nc.vector.tensor_scalar_max(stage2[:], stage2[:], 0.0)
        nc.sync.dma_start(out_d[:], stage2[:])

    nc.compile()
    return nc


def _pack_inputs(input, W1, W2, W3, b1, b2, b3):
    # Host-materialized im2col: Xrep[b, k, t] = x[b, t + k//300, k%300],
    # laid out as 12 K-tiles of 128 rows, SEQ padded to 400 with zeros.
    xt = np.zeros((B, EMB, SEQP), np.float32)
    xt[:, :, :SEQ] = np.asarray(input, np.float32).transpose(0, 2, 1)
    xrep = np.zeros((B, KTOT * 128, SEQP), np.float32)
    for j in range(5):
        rows = xrep[:, j * EMB : (j + 1) * EMB, : SEQP - j]
        rows[:] = xt[:, :, j:]
    xt = xrep.reshape(B, KTOT, 128, SEQP).astype(np.float16)

    wq = np.zeros((128, 2, NCOL, 128), np.float32)  # cast to fp16 below
    for br, (n, W) in enumerate(zip(NS, (W1, W2, W3))):
        Wt = np.asarray(W, np.float32).T  # [n*300, 256]
        for r in range(NTILES[br]):
            rows = Wt[128 * r : min(128 * (r + 1), n * EMB)]
            for dh in range(2):
                wq[: rows.shape[0], dh, COLB[br] + r, :] = (
                    rows[:, dh * 128 : (dh + 1) * 128]
                )

    wq = wq.astype(np.float16)

    bp = np.empty((128, 3, 2), np.float32)
    for br, b in enumerate((b1, b2, b3)):
        b = np.asarray(b, np.float32).reshape(DEPTH)
        for dh in range(2):
            bp[:, br, dh] = b[dh * 128 : (dh + 1) * 128]
    return xt, wq, bp


def kernel(input, W1, W2, W3, b1, b2, b3):
    global _built, LAST_RESULT
    from concourse.bass_utils import run_bass_kernel_spmd

    xt, wq, bp = _pack_inputs(input, W1, W2, W3, b1, b2, b3)

    if _built is None:
        _built = _build_bass()
    nc = _built

    in_maps = [
        {"xt": xt[c * BPC : (c + 1) * BPC], "wq": wq, "bp": bp}
        for c in range(NCORES)
    ]
    res = run_bass_kernel_spmd(
        nc, in_maps, core_ids=list(range(NCORES)), trace=TRACE
    )
    LAST_RESULT = res

    out = np.empty((B, 3 * DEPTH), np.float32)
    for c in range(NCORES):
        arr = res.results[c]["out_t"]  # [128, 3, 2, BPC]
        out[c * BPC : (c + 1) * BPC] = arr.transpose(3, 1, 2, 0).reshape(BPC, 768)
    return out



# revision 2
# speedup vs baseline: 1.3699x; 1.3699x over previous
"""TextCNN-style conv layer (kernel sizes 3/4/5, EMB=300 -> DEPTH=256, bias,
ReLU, max-pool over time) as a Bass/Tile kernel for 8 Trainium2 NeuronCores.

Strategy: data-parallel over batch (8 samples per core), weights replicated.

Conv as dense-K matmuls over a host-materialized im2col matrix
Xrep[k, t] = x[t + k//300, k%300], shared by all three branches (branch n
reads rows [0, n*300), its weights zero-padded to the K-tile boundary).

fp8 e4m3 + DoubleRow: the PE virtualizes to 128x256, contracting 256 rows
per matmul (2 fp8 weights per cell), so each branch needs ceil(n*300/256)
K-tiles: 4/5/6 -> 15 matmuls per sample per depth-half vs 30 at K=128.
Both operands quantize to e4m3; the measured end-to-end L2 error vs the
fp32 reference is ~1.1e-2 (accumulation stays fp32 in PSUM).

Schedule: group-major over (branch, half) with the sample loop innermost,
so one weight tile serves 8 consecutive matmuls (amortized LDWEIGHTS) and
the 8 samples' accumulations ride the 8 PSUM banks concurrently.  The
whole fp8 working set (4.9MB im2col + 1MB weights per core) stays
resident in SBUF; x tiles stream r-major so the first groups' data lands
first.

Epilogue: relu(max_i(y + b)) == max(0, max_i y + b): DVE reduce_max over
the window axis straight out of PSUM, broadcast bias add + clamp at 0,
output staged [d, branch, half, sample] per core and de-transposed on
host.
"""

import numpy as np
import ml_dtypes

B, SEQ, EMB = 64, 394, 300
DEPTH = 256
NCORES = 8
BPC = B // NCORES  # samples per core
SEQP = 400  # x_t free-dim padded (zeros) so shifted loads stay in bounds
NS = (3, 4, 5)
NT8 = (4, 5, 6)  # ceil(n*300/256) 256-row K-tiles per branch
KTOT8 = 6  # distinct Xrep K-tiles (256 rows) per sample
KROWS = 256 * KTOT8  # 1536 (1500 real + 36 zero rows)

TRACE = False
LAST_RESULT = None

_built = None


def _build_bass():
    import concourse.mybir as mybir
    import concourse.tile as tile
    from concourse import bacc
    from contextlib import ExitStack

    f32 = mybir.dt.float32
    f8 = mybir.dt.float8e4
    DR = mybir.MatmulPerfMode.DoubleRow

    nc = bacc.Bacc("TRN2", target_bir_lowering=False)
    xt_d = nc.dram_tensor(
        "xt", (BPC * KTOT8, 128, 2, SEQP), f8, kind="ExternalInput"
    )
    w_d = {
        (dh, br): nc.dram_tensor(
            f"w{dh}{br}", (128, NT8[br], 2, 128), f8, kind="ExternalInput"
        )
        for dh in range(2)
        for br in range(3)
    }
    bp_d = nc.dram_tensor("bp", (128, 3, 2), f32, kind="ExternalInput")
    out_d = nc.dram_tensor("out_t", (128, 3, 2, BPC), f32, kind="ExternalOutput")

    with tile.TileContext(nc) as tc, ExitStack() as ctx:
        xpool = ctx.enter_context(tc.tile_pool(name="x", bufs=1))
        wpool = ctx.enter_context(tc.tile_pool(name="w", bufs=1))
        cpool = ctx.enter_context(tc.tile_pool(name="consts", bufs=1))
        spool = ctx.enter_context(tc.tile_pool(name="stage", bufs=1))
        pspool = ctx.enter_context(tc.tile_pool(name="ps", bufs=8, space="PSUM"))

        # x segments alternate over the two fast HWDGE rings (SP, ACT);
        # most weights + bias go on the gpsimd SWDGE ring in parallel so
        # they never block the x pipeline.
        hw_engines = (nc.sync, nc.scalar)

        wts = {}

        def load_w(dh, br, eng):
            wt = wpool.tile(
                [128, NT8[br], 2, 128], f8, tag=f"w{dh}{br}", name=f"w{dh}{br}"
            )
            eng.dma_start(wt[:], w_d[dh, br][:])
            wts[dh, br] = wt

        # First group's weights go ahead of x on the sync ring.
        load_w(0, 0, nc.sync)

        xrs = [[None] * KTOT8 for _ in range(BPC)]
        rr = 0
        for r in range(KTOT8):
            for s in range(BPC):
                t = xpool.tile(
                    [128, 2, SEQP], f8, tag=f"x{s}_{r}", name=f"x{s}_{r}"
                )
                hw_engines[rr % 2].dma_start(t[:], xt_d[s * KTOT8 + r])
                rr += 1
                xrs[s][r] = t
            if r == 0:
                load_w(1, 0, nc.gpsimd)
            elif r == 1:
                load_w(0, 1, nc.gpsimd)
                load_w(1, 1, nc.gpsimd)
            elif r == 2:
                load_w(0, 2, nc.gpsimd)
                load_w(1, 2, nc.gpsimd)

        bt = cpool.tile([128, 3, 2], f32)
        nc.gpsimd.dma_start(bt[:], bp_d[:])

        stage = spool.tile([128, 3, 2, BPC], f32)

        # br outer so the second group (dh=1) reuses the x tiles the first
        # just consumed, giving the DMA ring slack to stay ahead.
        for br in range(3):
            nt = NT8[br]
            nw = SEQ - NS[br]  # windows the reference maxes over
            for dh in range(2):
                pss = [
                    pspool.tile(
                        [128, 512], f32, tag="ps", name=f"ps_{dh}_{br}_{s}"
                    )
                    for s in range(BPC)
                ]
                for r in range(nt):
                    for s in range(BPC):
                        nc.tensor.matmul(
                            pss[s][:, :nw],
                            lhsT=wts[dh, br][:, r, :, :],
                            rhs=xrs[s][r][:, :, :nw],
                            start=(r == 0),
                            stop=(r == nt - 1),
                            perf_mode=DR,
                        )
                for s in range(BPC):
                    nc.vector.reduce_max(
                        stage[:, br, dh, s : s + 1],
                        pss[s][:, :nw],
                        axis=mybir.AxisListType.X,
                    )

        stage2 = spool.tile([128, 3, 2, BPC], f32)
        nc.vector.tensor_tensor(
            stage2[:],
            stage[:],
            bt[:, :, :, None].to_broadcast((128, 3, 2, BPC)),
            mybir.AluOpType.add,
        )
        nc.vector.tensor_scalar_max(stage2[:], stage2[:], 0.0)
        nc.sync.dma_start(out_d[:], stage2[:])

    nc.compile()
    return nc


def _pack_inputs(input, W1, W2, W3, b1, b2, b3):
    f8 = ml_dtypes.float8_e4m3

    # Host-materialized im2col: Xrep[b, k, t] = x[b, t + k//300, k%300],
    # SEQ padded to 400 with zeros, K padded to 1536 with zero rows.
    xt = np.zeros((B, EMB, SEQP), np.float32)
    xt[:, :, :SEQ] = np.asarray(input, np.float32).transpose(0, 2, 1)
    xrep = np.zeros((B, KROWS, SEQP), np.float32)
    for j in range(5):
        xrep[:, j * EMB : (j + 1) * EMB, : SEQP - j] = xt[:, :, j:]
    # global row c = 256r + 128i + p  ->  [b, r, p, i, t]
    x8 = (
        xrep.reshape(B, KTOT8, 2, 128, SEQP)
        .transpose(0, 1, 3, 2, 4)
        .astype(f8)
    )  # [B, 6, 128, 2, 400]

    ws = {}
    for br, (n, W) in enumerate(zip(NS, (W1, W2, W3))):
        Wp = np.zeros((KROWS, DEPTH), np.float32)
        Wp[: n * EMB] = np.asarray(W, np.float32).T
        v = Wp.reshape(KTOT8, 2, 128, 2, 128)  # (r, i, p, dh, m)
        for dh in range(2):
            ws[dh, br] = np.ascontiguousarray(
                v[: NT8[br], :, :, dh, :].transpose(2, 0, 1, 3)
            ).astype(f8)  # (p, r, i, m)

    bp = np.empty((128, 3, 2), np.float32)
    for br, b in enumerate((b1, b2, b3)):
        b = np.asarray(b, np.float32).reshape(DEPTH)
        for dh in range(2):
            bp[:, br, dh] = b[dh * 128 : (dh + 1) * 128]
    return x8, ws, bp


def kernel(input, W1, W2, W3, b1, b2, b3):
    global _built, LAST_RESULT
    from concourse.bass_utils import run_bass_kernel_spmd

    x8, ws, bp = _pack_inputs(input, W1, W2, W3, b1, b2, b3)

    if _built is None:
        _built = _build_bass()
    nc = _built

    in_maps = []
    for c in range(NCORES):
        m = {
            "xt": np.ascontiguousarray(
                x8[c * BPC : (c + 1) * BPC]
            ).reshape(BPC * KTOT8, 128, 2, SEQP),
            "bp": bp,
        }
        for dh in range(2):
            for br in range(3):
                m[f"w{dh}{br}"] = ws[dh, br]
        in_maps.append(m)

    res = run_bass_kernel_spmd(
        nc, in_maps, core_ids=list(range(NCORES)), trace=TRACE
    )
    LAST_RESULT = res

    out = np.empty((B, 3 * DEPTH), np.float32)
    for c in range(NCORES):
        arr = res.results[c]["out_t"]  # [128, 3, 2, BPC]
        out[c * BPC : (c + 1) * BPC] = arr.transpose(3, 1, 2, 0).reshape(BPC, 768)
    return out


# revision 3
# speedup vs baseline: 1.5978x; 1.1664x over previous
"""TextCNN-style conv layer (kernel sizes 3/4/5, EMB=300 -> DEPTH=256, bias,
ReLU, max-pool over time) as a Bass/Tile kernel for 8 Trainium2 NeuronCores.

Strategy: data-parallel over batch (8 samples per core), weights replicated.

Conv as dense-K matmuls over a host-materialized im2col matrix
Xrep[k, t] = x[t + k//300, k%300], shared by all three branches (branch n
reads rows [0, n*300), its weights zero-padded to the K-tile boundary).

fp8 e4m3 + DoubleRow: the PE virtualizes to 128x256, contracting 256 rows
per matmul (2 fp8 weights per cell), so each branch needs ceil(n*300/256)
K-tiles: 4/5/6 -> 15 matmuls per sample per depth-half vs 30 at K=128.
Both operands quantize to e4m3; measured end-to-end L2 error vs the fp32
reference is ~1.2e-2 (accumulation stays fp32 in PSUM).

Schedule: passes over (branch, sample-group-of-4) with r outer and
(half, sample) inner, so the 8 concurrent accumulations (2 halves x 4
samples) exactly fill the 8 PSUM banks, each weight tile serves 4
consecutive matmuls (LDWEIGHTS hides under the matmul stream), and each
x tile feeds both depth-halves -- halving the front-edge DMA demand so
the PE never starves (v1 starved for ~20us and the HAM clock gate kept
the PE at 1.2GHz).  x streams as 12 batched DMAs (one per K-tile x
sample-half, 410KB each) instead of 48 small ones: a dma_start costs
~0.7us of issuing-engine time, so small DMAs serialize.  A short stream
of dummy matmuls on a memset tile warms the PE clock gate during the
initial DMA ramp.

Epilogue: relu(max_i(y + b)) == max(0, max_i y + b): DVE reduce_max over
the window axis straight out of PSUM, broadcast bias add + clamp at 0,
output staged [d, branch, half, sample] per core and de-transposed on
host.
"""

import numpy as np
import ml_dtypes

B, SEQ, EMB = 64, 394, 300
DEPTH = 256
NCORES = 8
BPC = B // NCORES  # samples per core
SEQP = 400  # x_t free-dim padded (zeros) so shifted loads stay in bounds
NS = (3, 4, 5)
NT8 = (4, 5, 6)  # ceil(n*300/256) 256-row K-tiles per branch
KTOT8 = 6  # distinct Xrep K-tiles (256 rows) per sample
KROWS = 256 * KTOT8  # 1536 (1500 real + 36 zero rows)
NWARM = 15  # PE clock-gate warmup matmuls

TRACE = False
LAST_RESULT = None

_built = None


def _build_bass():
    import concourse.mybir as mybir
    import concourse.tile as tile
    from concourse import bacc
    from contextlib import ExitStack

    f32 = mybir.dt.float32
    f8 = mybir.dt.float8e4
    DR = mybir.MatmulPerfMode.DoubleRow

    nc = bacc.Bacc("TRN2", target_bir_lowering=False)
    # index r*2 + half; dim2 is (sample-in-half, i) fused
    xt_d = nc.dram_tensor(
        "xt", (KTOT8 * 2, 128, 8, SEQP), f8, kind="ExternalInput"
    )
    w_d = {
        (dh, br): nc.dram_tensor(
            f"w{dh}{br}", (128, NT8[br], 2, 128), f8, kind="ExternalInput"
        )
        for dh in range(2)
        for br in range(3)
    }
    bp_d = nc.dram_tensor("bp", (128, 3, 2), f32, kind="ExternalInput")
    out_d = nc.dram_tensor("out_t", (128, 3, 2, BPC), f32, kind="ExternalOutput")

    with tile.TileContext(nc) as tc, ExitStack() as ctx:
        xpool = ctx.enter_context(tc.tile_pool(name="x", bufs=1))
        wpool = ctx.enter_context(tc.tile_pool(name="w", bufs=1))
        cpool = ctx.enter_context(tc.tile_pool(name="consts", bufs=1))
        spool = ctx.enter_context(tc.tile_pool(name="stage", bufs=1))
        pspool = ctx.enter_context(tc.tile_pool(name="ps", bufs=8, space="PSUM"))

        wts = {}

        def load_w(dh, br, eng):
            wt = wpool.tile(
                [128, NT8[br], 2, 128], f8, tag=f"w{dh}{br}", name=f"w{dh}{br}"
            )
            eng.dma_start(wt[:], w_d[dh, br][:])
            wts[dh, br] = wt

        # PE clock-gate warmup operand: a memset tile, ready ~immediately.
        wu = cpool.tile([128, 2, 128], f8)
        nc.gpsimd.memset(wu[:], 0)

        # First pass reads (dh0|dh1, r0, s0-3): w00 leads the sync ring, w10
        # the scalar ring, then x streams r-major with the s0-3 half on sync
        # (feeding sample-group 0) and s4-7 on scalar.  Remaining weights +
        # bias ride the gpsimd SWDGE ring in parallel.
        load_w(0, 0, nc.sync)
        load_w(1, 0, nc.scalar)
        xh = {}
        for r in range(KTOT8):
            for h, eng in ((0, nc.sync), (1, nc.scalar)):
                t = xpool.tile(
                    [128, 8, SEQP], f8, tag=f"x{r}_{h}", name=f"x{r}_{h}"
                )
                eng.dma_start(t[:], xt_d[r * 2 + h])
                xh[r, h] = t
            if r == 0:
                load_w(0, 1, nc.gpsimd)
                load_w(1, 1, nc.gpsimd)
            elif r == 1:
                load_w(0, 2, nc.gpsimd)
                load_w(1, 2, nc.gpsimd)

        bt = cpool.tile([128, 3, 2], f32)
        nc.gpsimd.dma_start(bt[:], bp_d[:])

        # Warm the HAM clock gate while the first DMAs land (PE would
        # otherwise sit idle and start cold at 1.2GHz).
        ps_wu = pspool.tile([128, 512], f32, tag="ps", name="ps_wu")
        for k in range(NWARM):
            nc.tensor.matmul(
                ps_wu[:, :128],
                lhsT=wu[:],
                rhs=wu[:],
                start=True,
                stop=True,
                perf_mode=DR,
            )

        stage = spool.tile([128, 3, 2, BPC], f32)

        for br in range(3):
            nt = NT8[br]
            nw = SEQ - NS[br]  # windows the reference maxes over
            for sg in range(2):
                pss = {
                    (dh, j): pspool.tile(
                        [128, 512], f32, tag="ps", name=f"ps_{br}_{sg}_{dh}_{j}"
                    )
                    for dh in range(2)
                    for j in range(4)
                }
                for r in range(nt):
                    for dh in range(2):
                        for j in range(4):
                            s = sg * 4 + j
                            nc.tensor.matmul(
                                pss[dh, j][:, :nw],
                                lhsT=wts[dh, br][:, r, :, :],
                                rhs=xh[r, sg][:, 2 * j : 2 * j + 2, :nw],
                                start=(r == 0),
                                stop=(r == nt - 1),
                                perf_mode=DR,
                            )
                for dh in range(2):
                    for j in range(4):
                        s = sg * 4 + j
                        nc.vector.reduce_max(
                            stage[:, br, dh, s : s + 1],
                            pss[dh, j][:, :nw],
                            axis=mybir.AxisListType.X,
                        )

        stage2 = spool.tile([128, 3, 2, BPC], f32)
        nc.vector.tensor_tensor(
            stage2[:],
            stage[:],
            bt[:, :, :, None].to_broadcast((128, 3, 2, BPC)),
            mybir.AluOpType.add,
        )
        nc.vector.tensor_scalar_max(stage2[:], stage2[:], 0.0)
        nc.sync.dma_start(out_d[:], stage2[:])

    nc.compile()
    return nc


def _pack_inputs(input, W1, W2, W3, b1, b2, b3):
    f8 = ml_dtypes.float8_e4m3

    # Host-materialized im2col: Xrep[b, k, t] = x[b, t + k//300, k%300],
    # SEQ padded to 400 with zeros, K padded to 1536 with zero rows.
    xt = np.zeros((B, EMB, SEQP), np.float32)
    xt[:, :, :SEQ] = np.asarray(input, np.float32).transpose(0, 2, 1)
    xrep = np.zeros((B, KROWS, SEQP), np.float32)
    for j in range(5):
        xrep[:, j * EMB : (j + 1) * EMB, : SEQP - j] = xt[:, :, j:]
    # global row c = 256r + 128i + p  ->  [b, r, p, i, t]
    x8 = (
        xrep.reshape(B, KTOT8, 2, 128, SEQP)
        .transpose(0, 1, 3, 2, 4)
        .astype(f8)
    )  # [B, 6, 128, 2, 400]

    ws = {}
    for br, (n, W) in enumerate(zip(NS, (W1, W2, W3))):
        Wp = np.zeros((KROWS, DEPTH), np.float32)
        Wp[: n * EMB] = np.asarray(W, np.float32).T
        v = Wp.reshape(KTOT8, 2, 128, 2, 128)  # (r, i, p, dh, m)
        for dh in range(2):
            ws[dh, br] = np.ascontiguousarray(
                v[: NT8[br], :, :, dh, :].transpose(2, 0, 1, 3)
            ).astype(f8)  # (p, r, i, m)

    bp = np.empty((128, 3, 2), np.float32)
    for br, b in enumerate((b1, b2, b3)):
        b = np.asarray(b, np.float32).reshape(DEPTH)
        for dh in range(2):
            bp[:, br, dh] = b[dh * 128 : (dh + 1) * 128]
    return x8, ws, bp


def kernel(input, W1, W2, W3, b1, b2, b3):
    global _built, LAST_RESULT
    from concourse.bass_utils import run_bass_kernel_spmd

    x8, ws, bp = _pack_inputs(input, W1, W2, W3, b1, b2, b3)

    if _built is None:
        _built = _build_bass()
    nc = _built

    in_maps = []
    for c in range(NCORES):
        cx = x8[c * BPC : (c + 1) * BPC]  # [8, 6, 128, 2, 400] (s, r, p, i, t)
        # -> [r, half, p, (s-in-half, i), t]
        cx = (
            cx.reshape(2, 4, KTOT8, 128, 2, SEQP)
            .transpose(2, 0, 3, 1, 4, 5)
            .reshape(KTOT8 * 2, 128, 8, SEQP)
        )
        m = {"xt": np.ascontiguousarray(cx), "bp": bp}
        for dh in range(2):
            for br in range(3):
                m[f"w{dh}{br}"] = ws[dh, br]
        in_maps.append(m)

    res = run_bass_kernel_spmd(
        nc, in_maps, core_ids=list(range(NCORES)), trace=TRACE
    )
    LAST_RESULT = res

    out = np.empty((B, 3 * DEPTH), np.float32)
    for c in range(NCORES):
        arr = res.results[c]["out_t"]  # [128, 3, 2, BPC]
        out[c * BPC : (c + 1) * BPC] = arr.transpose(3, 1, 2, 0).reshape(BPC, 768)
    return out


# revision 9
# speedup vs baseline: 1.7362x; 1.0867x over previous
"""TextCNN-style conv layer (kernel sizes 3/4/5, EMB=300 -> DEPTH=256, bias,
ReLU, max-pool over time) as a Bass/Tile kernel for 8 Trainium2 NeuronCores.

Strategy: data-parallel over batch (8 samples per core), weights replicated.

Conv as dense-K matmuls over a host-materialized im2col matrix
Xrep[k, t] = x[t + k//300, k%300], shared by all three branches (branch n
reads rows [0, n*300), its weights zero-padded to the K-tile boundary).

fp8 e4m3 + DoubleRow: the PE virtualizes to 128x256, contracting 256 rows
per matmul (2 fp8 weights per cell), so each branch needs ceil(n*300/256)
K-tiles: 4/5/6 -> 15 matmuls per sample per depth-half vs 30 at K=128.
Both operands quantize to e4m3; measured end-to-end L2 error vs the fp32
reference is ~1.2e-2 (accumulation stays fp32 in PSUM).

Schedule: sample-group-of-4 OUTER, then branch, then r, then (half,
sample) inner: the 8 concurrent accumulations (2 halves x 4 samples)
exactly fill the 8 PSUM banks, each weight tile serves 4 consecutive
matmuls (LDWEIGHTS hides under the matmul stream), and each x tile is
consumed by both depth-halves AND all three branches before the next
sample-group needs fresh data -- so the steady DMA demand (~124GB/s) is
far under the 358GB/s HBM rate and only the first pass's front edge is
tight.  That front edge streams as 2-sample tiles split across BOTH
HWDGE rings in parallel (the two rings share SDMA bandwidth, so one
ring alone delivers a tile at ~half rate); the second sample-group's
tiles follow as full 4-sample DMAs.  A stream of dummy matmuls on a
memset tile warms the PE HAM clock gate (1.2 -> 2.4GHz after ~3.4us of
sustained activity) during the initial DMA ramp, timed to end just as
the first real data lands.

Epilogue: relu(max_i(y + b)) == max(0, max_i y + b): DVE reduce_max over
the window axis straight out of PSUM, broadcast bias add + clamp at 0,
output staged [d, branch, half, sample] per core and de-transposed on
host.
"""

import numpy as np
import ml_dtypes

B, SEQ, EMB = 64, 394, 300
DEPTH = 256
NCORES = 8
BPC = B // NCORES  # samples per core
SEQP = 400  # x_t free-dim padded (zeros) so shifted loads stay in bounds
NS = (3, 4, 5)
NT8 = (4, 5, 6)  # ceil(n*300/256) 256-row K-tiles per branch
KTOT8 = 6  # distinct Xrep K-tiles (256 rows) per sample
KROWS = 256 * KTOT8  # 1536 (1500 real + 36 zero rows)
NWARM = 26  # PE clock-gate warmup matmuls (~3.6us at 1.2GHz)

TRACE = False
LAST_RESULT = None

_built = None


def _build_bass():
    import concourse.mybir as mybir
    import concourse.tile as tile
    from concourse import bacc
    from contextlib import ExitStack

    f32 = mybir.dt.float32
    f8 = mybir.dt.float8e4
    DR = mybir.MatmulPerfMode.DoubleRow

    nc = bacc.Bacc("TRN2", target_bir_lowering=False)
    # sample-group 0 (s0-3) as 2-sample tiles, index r*2 + pair; dim2 is
    # (sample-in-pair, i) fused
    xa_d = nc.dram_tensor(
        "xa", (KTOT8 * 2, 128, 4, SEQP), f8, kind="ExternalInput"
    )
    # sample-group 1 (s4-7) as 4-sample tiles, index r; dim2 is (sample, i)
    xb_d = nc.dram_tensor("xb", (KTOT8, 128, 8, SEQP), f8, kind="ExternalInput")
    w_d = {
        (dh, br): nc.dram_tensor(
            f"w{dh}{br}", (128, NT8[br], 2, 128), f8, kind="ExternalInput"
        )
        for dh in range(2)
        for br in range(3)
    }
    bp_d = nc.dram_tensor("bp", (128, 3, 2), f32, kind="ExternalInput")
    out_d = nc.dram_tensor("out_t", (128, 3, 2, BPC), f32, kind="ExternalOutput")

    with tile.TileContext(nc) as tc, ExitStack() as ctx:
        xpool = ctx.enter_context(tc.tile_pool(name="x", bufs=1))
        wpool = ctx.enter_context(tc.tile_pool(name="w", bufs=1))
        cpool = ctx.enter_context(tc.tile_pool(name="consts", bufs=1))
        spool = ctx.enter_context(tc.tile_pool(name="stage", bufs=1))
        pspool = ctx.enter_context(tc.tile_pool(name="ps", bufs=8, space="PSUM"))

        wts = {}

        def load_w(dh, br, eng):
            wt = wpool.tile(
                [128, NT8[br], 2, 128], f8, tag=f"w{dh}{br}", name=f"w{dh}{br}"
            )
            eng.dma_start(wt[:], w_d[dh, br][:])
            wts[dh, br] = wt

        # PE clock-gate warmup operand: a memset tile, ready ~immediately.
        wu = cpool.tile([128, 2, 128], f8)
        nc.gpsimd.memset(wu[:], 0)

        # w00 leads the sync ring, w10 the scalar ring; the first phase's x
        # (s0-3) streams as 2-sample tiles alternating across both rings in
        # exact need order (the rings share SDMA bandwidth, so the pair
        # halves of one r-tile drain in parallel); the second phase's s4-7
        # tiles follow.  Remaining weights + bias on the gpsimd SWDGE ring.
        load_w(0, 0, nc.sync)
        load_w(1, 0, nc.scalar)
        xa, xb = {}, {}
        for r in range(KTOT8):
            for c, eng in ((0, nc.sync), (1, nc.scalar)):
                t = xpool.tile(
                    [128, 4, SEQP], f8, tag=f"xa{r}_{c}", name=f"xa{r}_{c}"
                )
                eng.dma_start(t[:], xa_d[r * 2 + c])
                xa[r, c] = t
            if r == 0:
                load_w(0, 1, nc.gpsimd)
                load_w(1, 1, nc.gpsimd)
            elif r == 1:
                load_w(0, 2, nc.gpsimd)
                load_w(1, 2, nc.gpsimd)
        for r in range(KTOT8):
            t = xpool.tile([128, 8, SEQP], f8, tag=f"xb{r}", name=f"xb{r}")
            (nc.sync, nc.scalar)[r % 2].dma_start(t[:], xb_d[r])
            xb[r] = t

        bt = cpool.tile([128, 3, 2], f32)
        nc.gpsimd.dma_start(bt[:], bp_d[:])

        # Warm the HAM clock gate while the first DMAs land (PE would
        # otherwise sit idle and start cold at 1.2GHz).
        ps_wu = pspool.tile([128, 512], f32, tag="ps", name="ps_wu")
        for k in range(NWARM):
            nc.tensor.matmul(
                ps_wu[:, :128],
                lhsT=wu[:],
                rhs=wu[:],
                start=True,
                stop=True,
                perf_mode=DR,
            )

        stage = spool.tile([128, 3, 2, BPC], f32)

        for sg in range(2):
            for br in range(3):
                nt = NT8[br]
                nw = SEQ - NS[br]  # windows the reference maxes over
                pss = {
                    (dh, j): pspool.tile(
                        [128, 512], f32, tag="ps", name=f"ps_{br}_{sg}_{dh}_{j}"
                    )
                    for dh in range(2)
                    for j in range(4)
                }
                for r in range(nt):
                    for dh in range(2):
                        for j in range(4):
                            if sg == 0:
                                rhs = xa[r, j // 2][
                                    :, 2 * (j % 2) : 2 * (j % 2) + 2, :nw
                                ]
                            else:
                                rhs = xb[r][:, 2 * j : 2 * j + 2, :nw]
                            nc.tensor.matmul(
                                pss[dh, j][:, :nw],
                                lhsT=wts[dh, br][:, r, :, :],
                                rhs=rhs,
                                start=(r == 0),
                                stop=(r == nt - 1),
                                perf_mode=DR,
                            )
                for dh in range(2):
                    for j in range(4):
                        s = sg * 4 + j
                        nc.vector.reduce_max(
                            stage[:, br, dh, s : s + 1],
                            pss[dh, j][:, :nw],
                            axis=mybir.AxisListType.X,
                        )

        stage2 = spool.tile([128, 3, 2, BPC], f32)
        nc.vector.tensor_tensor(
            stage2[:],
            stage[:],
            bt[:, :, :, None].to_broadcast((128, 3, 2, BPC)),
            mybir.AluOpType.add,
        )
        nc.vector.tensor_scalar_max(stage2[:], stage2[:], 0.0)
        nc.sync.dma_start(out_d[:], stage2[:])

    nc.compile()
    return nc


def _pack_inputs(input, W1, W2, W3, b1, b2, b3):
    f8 = ml_dtypes.float8_e4m3

    # Host-materialized im2col: Xrep[b, k, t] = x[b, t + k//300, k%300],
    # SEQ padded to 400 with zeros, K padded to 1536 with zero rows.
    xt = np.zeros((B, EMB, SEQP), np.float32)
    xt[:, :, :SEQ] = np.asarray(input, np.float32).transpose(0, 2, 1)
    xrep = np.zeros((B, KROWS, SEQP), np.float32)
    for j in range(5):
        xrep[:, j * EMB : (j + 1) * EMB, : SEQP - j] = xt[:, :, j:]
    # global row c = 256r + 128i + p  ->  [b, r, p, i, t]
    x8 = (
        xrep.reshape(B, KTOT8, 2, 128, SEQP)
        .transpose(0, 1, 3, 2, 4)
        .astype(f8)
    )  # [B, 6, 128, 2, 400]

    ws = {}
    for br, (n, W) in enumerate(zip(NS, (W1, W2, W3))):
        Wp = np.zeros((KROWS, DEPTH), np.float32)
        Wp[: n * EMB] = np.asarray(W, np.float32).T
        v = Wp.reshape(KTOT8, 2, 128, 2, 128)  # (r, i, p, dh, m)
        for dh in range(2):
            ws[dh, br] = np.ascontiguousarray(
                v[: NT8[br], :, :, dh, :].transpose(2, 0, 1, 3)
            ).astype(f8)  # (p, r, i, m)

    bp = np.empty((128, 3, 2), np.float32)
    for br, b in enumerate((b1, b2, b3)):
        b = np.asarray(b, np.float32).reshape(DEPTH)
        for dh in range(2):
            bp[:, br, dh] = b[dh * 128 : (dh + 1) * 128]
    return x8, ws, bp


def kernel(input, W1, W2, W3, b1, b2, b3):
    global _built, LAST_RESULT
    from concourse.bass_utils import run_bass_kernel_spmd

    x8, ws, bp = _pack_inputs(input, W1, W2, W3, b1, b2, b3)

    if _built is None:
        _built = _build_bass()
    nc = _built

    in_maps = []
    for c in range(NCORES):
        cx = x8[c * BPC : (c + 1) * BPC]  # [8, 6, 128, 2, 400] (s, r, p, i, t)
        # s0-3 -> [r, pair, p, (s-in-pair, i), t]
        xa = (
            cx[:4]
            .reshape(2, 2, KTOT8, 128, 2, SEQP)  # (pair, s2, r, p, i, t)
            .transpose(2, 0, 3, 1, 4, 5)
            .reshape(KTOT8 * 2, 128, 4, SEQP)
        )
        # s4-7 -> [r, p, (s, i), t]
        xb = (
            cx[4:]
            .transpose(1, 2, 0, 3, 4)  # (r, p, s4, i, t)
            .reshape(KTOT8, 128, 8, SEQP)
        )
        m = {
            "xa": np.ascontiguousarray(xa),
            "xb": np.ascontiguousarray(xb),
            "bp": bp,
        }
        for dh in range(2):
            for br in range(3):
                m[f"w{dh}{br}"] = ws[dh, br]
        in_maps.append(m)

    res = run_bass_kernel_spmd(
        nc, in_maps, core_ids=list(range(NCORES)), trace=TRACE
    )
    LAST_RESULT = res

    out = np.empty((B, 3 * DEPTH), np.float32)
    for c in range(NCORES):
        arr = res.results[c]["out_t"]  # [128, 3, 2, BPC]
        out[c * BPC : (c + 1) * BPC] = arr.transpose(3, 1, 2, 0).reshape(BPC, 768)
    return out
